# revision 2
# baseline (speedup 1.0000x reference)
"""GNN message-passing kernel for Trainium2 (8 NeuronCores).

Gathers use InstDMAGatherAnt (vectorized Q7 descriptor generation, one
instruction per chunk x bank) from a 256B-strided replicated g-table in
DRAM, instead of one indirect DMA per 128-row round (the v1 baseline pays
~1us of serialized SWDGE setup per round).  Table rows are banked
r -> (r % 4, r // 4) so local indices fit int16 (stride 1024B, base
offset bank*256B).  Slots per tile are padded per-bank to the
max-over-cores/partitions count so reduces stay rectangular; pad slots
gather a real row but carry edge weight 0.
"""
import sys
sys.path.insert(0, '/opt/trn_rl_repo')
import numpy as np

N = 100000
C = 16
LAYERS = 4
ALPHA = 0.5
N_CORES = 8
P = 128
PER_CORE = 12544            # 98 tiles of 128 (12500 real + 44 pad)
TILES = PER_CORE // P       # 98
NT = N_CORES * PER_CORE
NBANK = 4
NB_ROWS = NT // NBANK       # 25088 rows per bank (< 32768, int16-safe)
ROWW = 64                   # strided table row width in f32 (256B)
S_CH = 448                  # max msgs columns per chunk

_compiled = None


def _plan(K_tb):
    """K_tb: [TILES, NBANK] per-tile per-bank slot counts.
    Chunks of consecutive tiles with per-chunk-uniform bank widths
    Kb = max over chunk tiles. Returns list of
    (t0, nt, [K0..K3]) with nt * sum(Kb) <= S_CH."""
    chunks = []
    t = 0
    while t < TILES:
        nt = 1
        while t + nt <= TILES - 1 + 1:
            if t + nt > TILES - 1:
                break
            kb = K_tb[t:t + nt + 1].max(axis=0)
            if (nt + 1) * int(kb.sum()) > S_CH:
                break
            nt += 1
        kb = K_tb[t:t + nt].max(axis=0)
        chunks.append((t, nt, [int(v) for v in kb]))
        t += nt
    return chunks


def _dma_gather(eng, out_ap, in_ap, idxs_ap, num_idxs, elem_size, elem_step):
    """bass.dma_gather minus the 256B *payload* assert (the real HW
    constraint is on the row stride, which stays 256B-aligned here)."""
    import concourse.mybir as mybir
    from concourse import ap_utils
    assert idxs_ap.dtype == mybir.dt.int16
    assert in_ap.dtype == out_ap.dtype
    stride_bytes = elem_step * mybir.dt.size(in_ap.dtype)
    assert stride_bytes % 256 == 0 and stride_bytes // 256 < 256
    assert in_ap.ap[0][0] == elem_step
    assert ap_utils.ap_is_contiguous(in_ap.ap[1:])
    assert ap_utils.ap_is_contiguous(out_ap.ap[1:])
    assert ap_utils.ap_is_contiguous(idxs_ap.ap[1:])
    assert in_ap.ap[-1][1] == elem_size and out_ap.ap[-1][1] == elem_size
    assert out_ap.ap[0][1] * out_ap.ap[1][1] == num_idxs and num_idxs % 128 == 0
    _in_ap = eng.lower_ap_dma(in_ap, for_custom_bir_dma=True)
    _idxs_ap = eng.lower_ap(idxs_ap)
    _out_ap = eng.lower_ap(out_ap)
    return eng.add_instruction(mybir.InstDMAGatherAnt(
        name=eng.bass.get_next_instruction_name(),
        ins=[*_in_ap, _idxs_ap, eng.lower_val_access(eng.to_reg(num_idxs))],
        outs=[_out_ap], transpose=False,
        num_idxs=num_idxs, elem_size=elem_size,
        stride_bytes_256=stride_bytes // 256, gen_mode=0,
        single_packet=False, queue_num=0,
        sbuf_tokens_per_rank=0, sbuf_free_dim_per_rank=0,
        sbuf_free_dim_pad_per_rank=0, sbuf_byte_offset=0))


def _build(K_tb):
    import concourse.bass as bass
    import concourse.bacc as bacc
    import concourse.mybir as mybir
    from concourse.tile import TileContext

    AO = mybir.AluOpType
    f32 = mybir.dt.float32
    K_tb = np.asarray(K_tb)
    chunks = _plan(K_tb)
    total_cols = sum(nt * sum(kb) for (_, nt, kb) in chunks)
    idx_cols = [128 * nt * sum(kb) // 16 for (_, nt, kb) in chunks]
    max_cols = max(nt * sum(kb) for (_, nt, kb) in chunks)
    max_idx_cols = max(idx_cols)

    nc = bacc.Bacc("TRN2", target_bir_lowering=False, debug=False,
                   num_devices=N_CORES)

    h0_d = nc.dram_tensor("h0", [P, TILES, C], f32, kind="ExternalInput")
    w_d = nc.dram_tensor("w", [P, TILES], f32, kind="ExternalInput")
    d2_d = nc.dram_tensor("d2", [P, TILES], f32, kind="ExternalInput")
    gi_d = nc.dram_tensor("gi", [P, sum(idx_cols)], mybir.dt.int16,
                          kind="ExternalInput")
    ew_d = nc.dram_tensor("ew", [P, total_cols], f32, kind="ExternalInput")
    hout_d = nc.dram_tensor("hout", [P, TILES, C], f32,
                            kind="ExternalOutput")

    gslice_d = nc.dram_tensor("gslice", [PER_CORE, C], f32)
    gfull_c = nc.dram_tensor("gfullc", [NT, C], f32, addr_space="Shared")
    gfull_s = nc.dram_tensor("gfulls", [NT, ROWW], f32)
    rg = [list(range(N_CORES))]

    def bcast_last(ap, c):
        return bass.AP(ap.tensor, ap.offset, [*ap.ap, [0, c]])

    def g3(t):
        return t[:].rearrange("p (t c) -> p t c", c=C)

    with TileContext(nc) as tc:
        with tc.tile_pool(name="const", bufs=1) as cpool, \
             tc.tile_pool(name="gpool", bufs=3) as gpool:
            ew_t = cpool.tile([P, total_cols], f32)
            nc.sync.dma_start(out=ew_t[:], in_=ew_d[:])
            w_t = cpool.tile([P, TILES], f32)
            nc.sync.dma_start(out=w_t[:], in_=w_d[:])
            d2_t = cpool.tile([P, TILES], f32)
            nc.sync.dma_start(out=d2_t[:], in_=d2_d[:])
            h_t = cpool.tile([P, TILES * C], f32)
            nc.sync.dma_start(out=g3(h_t), in_=h0_d[:])
            g_t = cpool.tile([P, TILES * C], f32)
            agg_t = cpool.tile([P, TILES * C], f32)
            aggb = [cpool.tile([P, TILES * C], f32, name="aggb%d" % b)
                    for b in range(NBANK)]
            rs_t = cpool.tile([P, TILES], f32)

            for layer in range(LAYERS):
                nc.vector.tensor_tensor(
                    out=g3(g_t), in0=g3(h_t), in1=bcast_last(w_t[:], C),
                    op=AO.mult)
                nc.sync.dma_start(
                    out=gslice_d[:].rearrange("(p x) c -> p (x c)", p=P),
                    in_=g_t[:])
                nc.gpsimd.collective_compute(
                    "AllGather", AO.bypass,
                    ins=[gslice_d[:]], outs=[gfull_c[:]],
                    replica_groups=rg)
                # expand compact -> 256B-strided table (split: AP dims
                # are 16-bit, NT=100352 doesn't fit)
                nc.sync.dma_start(out=gfull_s[:NT // 2, 0:C],
                                  in_=gfull_c[:NT // 2])
                nc.sync.dma_start(out=gfull_s[NT // 2:, 0:C],
                                  in_=gfull_c[NT // 2:])

                colb = 0
                idxb = 0
                for ci, (t0, nt, kb) in enumerate(chunks):
                    W = sum(kb)
                    icols = idx_cols[ci]
                    idx_t = gpool.tile([P, max_idx_cols], mybir.dt.int16,
                                       tag="idx")
                    nc.sync.dma_start(out=idx_t[:, :icols],
                                      in_=gi_d[:, idxb:idxb + icols])
                    msgs = gpool.tile([P, max_cols * C], f32, tag="msgs")
                    cb = 0
                    ib = 0
                    for b in range(NBANK):
                        if kb[b] == 0:
                            continue
                        ncols = nt * kb[b]
                        nidx = 128 * ncols
                        src = bass.AP(gfull_s[:].tensor, b * ROWW,
                                      [[NBANK * ROWW, NB_ROWS], [1, C]])
                        _dma_gather(
                            nc.gpsimd,
                            out_ap=msgs[:, cb * C:(cb + ncols) * C]
                                .rearrange("p (s c) -> p s c", c=C),
                            in_ap=src,
                            idxs_ap=idx_t[:, ib:ib + nidx // 16],
                            num_idxs=nidx, elem_size=C, elem_step=NBANK * ROWW)
                        cb += ncols
                        ib += nidx // 16
                    nc.vector.tensor_tensor(
                        out=msgs[:, :W * nt * C].rearrange(
                            "p (s c) -> p s c", c=C),
                        in0=msgs[:, :W * nt * C].rearrange(
                            "p (s c) -> p s c", c=C),
                        in1=bcast_last(ew_t[:, colb:colb + W * nt], C),
                        op=AO.mult)
                    cb = 0
                    for b in range(NBANK):
                        if kb[b] == 0:
                            continue
                        nc.vector.tensor_reduce(
                            out=aggb[b][:, t0 * C:(t0 + nt) * C],
                            in_=msgs[:, cb * C:(cb + nt * kb[b]) * C]
                                .rearrange("p (t k c) -> p t c k",
                                           k=kb[b], c=C),
                            op=AO.add, axis=mybir.AxisListType.X)
                        cb += nt * kb[b]
                    colb += W * nt
                    idxb += icols

                nc.vector.tensor_tensor(out=agg_t[:], in0=aggb[0][:],
                                        in1=aggb[1][:], op=AO.add)
                nc.vector.tensor_tensor(out=agg_t[:], in0=agg_t[:],
                                        in1=aggb[2][:], op=AO.add)
                nc.vector.tensor_tensor(out=agg_t[:], in0=agg_t[:],
                                        in1=aggb[3][:], op=AO.add)
                # epilogue; d2 holds (1-ALPHA)*degree^2
                nc.vector.tensor_tensor(
                    out=g3(g_t), in0=g3(agg_t), in1=bcast_last(d2_t[:], C),
                    op=AO.mult)
                nc.vector.scalar_tensor_tensor(
                    out=g_t[:], in0=h_t[:], scalar=ALPHA, in1=g_t[:],
                    op0=AO.mult, op1=AO.add)
                nc.vector.tensor_reduce(
                    out=rs_t[:], in_=g3(g_t), op=AO.add,
                    axis=mybir.AxisListType.X)
                nc.vector.reciprocal(out=rs_t[:], in_=rs_t[:])
                nc.vector.tensor_tensor(
                    out=g3(h_t), in0=g3(g_t), in1=bcast_last(rs_t[:], C),
                    op=AO.mult)

            nc.sync.dma_start(out=hout_d[:], in_=g3(h_t))

    nc.compile()
    return nc, chunks


def _prep(x, W, edge_weight, degree, edge_index):
    src = edge_index[0].astype(np.int64)
    dst = edge_index[1].astype(np.int64)
    indeg = np.bincount(dst, minlength=N)

    pos_of = np.empty(N, dtype=np.int64)
    core_of = np.empty(N, dtype=np.int64)
    for cc in range(N_CORES):
        ids = np.arange(cc * 12500, (cc + 1) * 12500)
        order = ids[np.argsort(-indeg[ids], kind="stable")]
        core_of[order] = cc
        pos_of[order] = np.arange(12500)
    tile_of = pos_of // P
    part_of = pos_of % P
    row_of = core_of * PER_CORE + part_of * TILES + tile_of

    bank_of_row = row_of % NBANK
    loc_of_row = row_of // NBANK

    # per (core, part, tile, bank) counts -> K_tb = max over cores+parts
    eb = bank_of_row[src]
    key = ((core_of[dst] * P + part_of[dst]) * TILES + tile_of[dst]) * NBANK + eb
    cnt = np.bincount(key, minlength=N_CORES * P * TILES * NBANK)
    cnt = cnt.reshape(N_CORES, P, TILES, NBANK)
    K_tb = cnt.max(axis=(0, 1))                      # [TILES, NBANK]

    chunks = _plan(K_tb)
    # column base for (t, b): columns laid chunk-major, bank-major inside
    col_of = np.zeros((TILES, NBANK), dtype=np.int64)
    kb_of = np.zeros((TILES, NBANK), dtype=np.int64)
    colb = 0
    for (t0, nt, kb) in chunks:
        cb = colb
        for b in range(NBANK):
            for t in range(t0, t0 + nt):
                col_of[t, b] = cb + (t - t0) * kb[b]
                kb_of[t, b] = kb[b]
            cb += nt * kb[b]
        colb += nt * sum(kb)
    total_cols = colb

    # slot position for each edge: within-group rank
    order = np.argsort(key, kind="stable")
    ks = key[order]
    start = np.zeros(key.max() + 2, dtype=np.int64)
    cnts = np.bincount(ks)
    start[1:len(cnts) + 1] = np.cumsum(cnts)
    kslot = np.arange(len(ks)) - start[ks]

    ec, ep = core_of[dst[order]], part_of[dst[order]]
    et, ebk = tile_of[dst[order]], eb[order]
    col = col_of[et, ebk] + kslot

    ew_h = np.zeros((N_CORES, P, total_cols), dtype=np.float32)
    iv_h = np.zeros((N_CORES, P, total_cols), dtype=np.int16)
    ew_h[ec, ep, col] = edge_weight[order]
    iv_h[ec, ep, col] = loc_of_row[src[order]].astype(np.int16)

    # wrapped int16 idx tensors, concatenated per chunk/bank region
    gi_list = []
    for (t0, nt, kb) in chunks:
        for b in range(NBANK):
            if kb[b] == 0:
                continue
            c0 = col_of[t0, b]
            ncols = nt * kb[b]
            vals = iv_h[:, :, c0:c0 + ncols]             # [NC, P, ncols]
            # position i = col*128 + p ; wrapped [q, m] = value(m*16 + q%16)
            flat = vals.transpose(0, 2, 1).reshape(N_CORES, ncols * 128)
            wr = flat.reshape(N_CORES, ncols * 8, 16).transpose(0, 2, 1)
            wr = np.tile(wr, (1, 8, 1))                  # replicate to 128
            gi_list.append(wr)
    gi = np.concatenate(gi_list, axis=2)

    h0 = np.ones((N_CORES, P, TILES, C), dtype=np.float32)
    wv = np.zeros((N_CORES, P, TILES), dtype=np.float32)
    d2 = np.zeros((N_CORES, P, TILES), dtype=np.float32)
    h0[core_of, part_of, tile_of] = np.asarray(x)
    wv[core_of, part_of, tile_of] = np.asarray(W).reshape(-1)
    d2[core_of, part_of, tile_of] = (1.0 - ALPHA) * np.asarray(degree) ** 2
    return (K_tb, gi, ew_h, h0, wv, d2, core_of, part_of, tile_of,
            chunks, col_of, kb_of)


def kernel(x, W, edge_weight, degree, edge_index):
    global _compiled
    from concourse import bass_utils
    bass_utils.upload_artifacts = lambda tmpdir: "local://" + tmpdir

    pr = _prep(np.asarray(x), np.asarray(W), np.asarray(edge_weight),
               np.asarray(degree), np.asarray(edge_index))
    K_tb, gi, ew, h0, wv, d2, core_of, part_of, tile_of = pr[:9]

    key = tuple(int(v) for v in K_tb.ravel())
    if _compiled is None or _compiled[0] != key:
        _compiled = (key, _build(K_tb)[0])
    nc = _compiled[1]

    in_maps = [{"h0": h0[cc], "w": wv[cc], "d2": d2[cc],
                "gi": gi[cc], "ew": ew[cc]} for cc in range(N_CORES)]
    res = bass_utils.run_bass_kernel_spmd(
        nc, in_maps, core_ids=list(range(N_CORES)))
    houts = np.stack([res.results[cc]["hout"] for cc in range(N_CORES)])
    return houts[core_of, part_of, tile_of]


# revision 3
# speedup vs baseline: 1.0098x; 1.0098x over previous
"""GNN message-passing kernel for Trainium2 (8 NeuronCores).

Gathers use InstDMAGatherAnt (vectorized Q7 descriptor generation, one
instruction per chunk x bank) from a 256B-strided replicated g-table in
DRAM, instead of one indirect DMA per 128-row round (the v1 baseline pays
~1us of serialized SWDGE setup per round).  Table rows are banked
r -> (r % 4, r // 4) so local indices fit int16 (stride 1024B, base
offset bank*256B).  Slots per tile are padded per-bank to the
max-over-cores/partitions count so reduces stay rectangular; pad slots
gather a real row but carry edge weight 0.
"""
import sys
sys.path.insert(0, '/opt/trn_rl_repo')
import numpy as np

N = 100000
C = 16
LAYERS = 4
ALPHA = 0.5
N_CORES = 8
P = 128
PER_CORE = 12544            # 98 tiles of 128 (12500 real + 44 pad)
TILES = PER_CORE // P       # 98
NT = N_CORES * PER_CORE
NBANK = 4
NB_ROWS = NT // NBANK       # 25088 rows per bank (< 32768, int16-safe)
ROWW = 64                   # strided table row width in f32 (256B)
S_CH = 448                  # max msgs columns per chunk

_compiled = None


def _plan(K_tb):
    """K_tb: [TILES, NBANK] per-tile per-bank slot counts.
    Chunks of consecutive tiles with per-chunk-uniform bank widths
    Kb = max over chunk tiles. Returns list of
    (t0, nt, [K0..K3]) with nt * sum(Kb) <= S_CH."""
    chunks = []
    t = 0
    while t < TILES:
        nt = 1
        while t + nt <= TILES - 1 + 1:
            if t + nt > TILES - 1:
                break
            kb = K_tb[t:t + nt + 1].max(axis=0)
            if (nt + 1) * int(kb.sum()) > S_CH:
                break
            nt += 1
        kb = K_tb[t:t + nt].max(axis=0)
        chunks.append((t, nt, [int(v) for v in kb]))
        t += nt
    return chunks


def _dma_gather(eng, out_ap, in_ap, idxs_ap, num_idxs, elem_size, elem_step):
    """bass.dma_gather minus the 256B *payload* assert (the real HW
    constraint is on the row stride, which stays 256B-aligned here)."""
    import concourse.mybir as mybir
    from concourse import ap_utils
    assert idxs_ap.dtype == mybir.dt.int16
    assert in_ap.dtype == out_ap.dtype
    stride_bytes = elem_step * mybir.dt.size(in_ap.dtype)
    assert stride_bytes % 256 == 0 and stride_bytes // 256 < 256
    assert in_ap.ap[0][0] == elem_step
    assert ap_utils.ap_is_contiguous(in_ap.ap[1:])
    assert ap_utils.ap_is_contiguous(out_ap.ap[1:])
    assert ap_utils.ap_is_contiguous(idxs_ap.ap[1:])
    assert in_ap.ap[-1][1] == elem_size and out_ap.ap[-1][1] == elem_size
    assert out_ap.ap[0][1] * out_ap.ap[1][1] == num_idxs and num_idxs % 128 == 0
    _in_ap = eng.lower_ap_dma(in_ap, for_custom_bir_dma=True)
    _idxs_ap = eng.lower_ap(idxs_ap)
    _out_ap = eng.lower_ap(out_ap)
    return eng.add_instruction(mybir.InstDMAGatherAnt(
        name=eng.bass.get_next_instruction_name(),
        ins=[*_in_ap, _idxs_ap, eng.lower_val_access(eng.to_reg(num_idxs))],
        outs=[_out_ap], transpose=False,
        num_idxs=num_idxs, elem_size=elem_size,
        stride_bytes_256=stride_bytes // 256, gen_mode=0,
        single_packet=False, queue_num=0,
        sbuf_tokens_per_rank=0, sbuf_free_dim_per_rank=0,
        sbuf_free_dim_pad_per_rank=0, sbuf_byte_offset=0))


def _build(K_tb):
    import concourse.bass as bass
    import concourse.bacc as bacc
    import concourse.mybir as mybir
    from concourse.tile import TileContext

    AO = mybir.AluOpType
    f32 = mybir.dt.float32
    K_tb = np.asarray(K_tb)
    chunks = _plan(K_tb)
    total_cols = sum(nt * sum(kb) for (_, nt, kb) in chunks)
    idx_cols = [128 * nt * sum(kb) // 16 for (_, nt, kb) in chunks]
    max_cols = max(nt * sum(kb) for (_, nt, kb) in chunks)
    max_idx_cols = max(idx_cols)

    nc = bacc.Bacc("TRN2", target_bir_lowering=False, debug=False,
                   num_devices=N_CORES)

    h0_d = nc.dram_tensor("h0", [P, TILES, C], f32, kind="ExternalInput")
    w_d = nc.dram_tensor("w", [P, TILES], f32, kind="ExternalInput")
    d2_d = nc.dram_tensor("d2", [P, TILES], f32, kind="ExternalInput")
    gi_d = nc.dram_tensor("gi", [P, sum(idx_cols)], mybir.dt.int16,
                          kind="ExternalInput")
    ew_d = nc.dram_tensor("ew", [P, total_cols], f32, kind="ExternalInput")
    hout_d = nc.dram_tensor("hout", [P, TILES, C], f32,
                            kind="ExternalOutput")

    gslice_d = nc.dram_tensor("gslice", [PER_CORE, C], f32)
    gfull_c = nc.dram_tensor("gfullc", [NT, C], f32, addr_space="Shared")
    gfull_s = nc.dram_tensor("gfulls", [NT, ROWW], f32)
    rg = [list(range(N_CORES))]

    def bcast_last(ap, c):
        return bass.AP(ap.tensor, ap.offset, [*ap.ap, [0, c]])

    def g3(t):
        return t[:].rearrange("p (t c) -> p t c", c=C)

    with TileContext(nc) as tc:
        with tc.tile_pool(name="const", bufs=1) as cpool, \
             tc.tile_pool(name="gpool", bufs=3) as gpool:
            ew_t = cpool.tile([P, total_cols], f32)
            nc.sync.dma_start(out=ew_t[:], in_=ew_d[:])
            w_t = cpool.tile([P, TILES], f32)
            nc.sync.dma_start(out=w_t[:], in_=w_d[:])
            d2_t = cpool.tile([P, TILES], f32)
            nc.sync.dma_start(out=d2_t[:], in_=d2_d[:])
            h_t = cpool.tile([P, TILES * C], f32)
            nc.sync.dma_start(out=g3(h_t), in_=h0_d[:])
            g_t = cpool.tile([P, TILES * C], f32)
            agg_t = cpool.tile([P, TILES * C], f32)
            aggb = [cpool.tile([P, TILES * C], f32, name="aggb%d" % b)
                    for b in range(NBANK)]
            rs_t = cpool.tile([P, TILES], f32)

            for layer in range(LAYERS):
                nc.vector.tensor_tensor(
                    out=g3(g_t), in0=g3(h_t), in1=bcast_last(w_t[:], C),
                    op=AO.mult)
                nc.sync.dma_start(
                    out=gslice_d[:].rearrange("(p x) c -> p (x c)", p=P),
                    in_=g_t[:])
                nc.gpsimd.collective_compute(
                    "AllGather", AO.bypass,
                    ins=[gslice_d[:]], outs=[gfull_c[:]],
                    replica_groups=rg)
                # expand compact -> 256B-strided table, one copy per bank
                # (rows r = 4l+b) so bank-b gathers only wait on their
                # own expand; also keeps AP dims under the 16-bit limit
                for b in range(NBANK):
                    nc.sync.dma_start(
                        out=bass.AP(gfull_s[:].tensor, b * ROWW,
                                    [[NBANK * ROWW, NB_ROWS], [1, C]]),
                        in_=bass.AP(gfull_c[:].tensor, b * C,
                                    [[NBANK * C, NB_ROWS], [1, C]]))

                colb = 0
                idxb = 0
                for ci, (t0, nt, kb) in enumerate(chunks):
                    W = sum(kb)
                    icols = idx_cols[ci]
                    idx_t = gpool.tile([P, max_idx_cols], mybir.dt.int16,
                                       tag="idx")
                    nc.sync.dma_start(out=idx_t[:, :icols],
                                      in_=gi_d[:, idxb:idxb + icols])
                    msgs = gpool.tile([P, max_cols * C], f32, tag="msgs")
                    cb = 0
                    ib = 0
                    for b in range(NBANK):
                        if kb[b] == 0:
                            continue
                        ncols = nt * kb[b]
                        nidx = 128 * ncols
                        src = bass.AP(gfull_s[:].tensor, b * ROWW,
                                      [[NBANK * ROWW, NB_ROWS], [1, C]])
                        _dma_gather(
                            nc.gpsimd,
                            out_ap=msgs[:, cb * C:(cb + ncols) * C]
                                .rearrange("p (s c) -> p s c", c=C),
                            in_ap=src,
                            idxs_ap=idx_t[:, ib:ib + nidx // 16],
                            num_idxs=nidx, elem_size=C, elem_step=NBANK * ROWW)
                        cb += ncols
                        ib += nidx // 16
                    nc.vector.tensor_tensor(
                        out=msgs[:, :W * nt * C].rearrange(
                            "p (s c) -> p s c", c=C),
                        in0=msgs[:, :W * nt * C].rearrange(
                            "p (s c) -> p s c", c=C),
                        in1=bcast_last(ew_t[:, colb:colb + W * nt], C),
                        op=AO.mult)
                    cb = 0
                    for b in range(NBANK):
                        if kb[b] == 0:
                            continue
                        nc.vector.tensor_reduce(
                            out=aggb[b][:, t0 * C:(t0 + nt) * C],
                            in_=msgs[:, cb * C:(cb + nt * kb[b]) * C]
                                .rearrange("p (t k c) -> p t c k",
                                           k=kb[b], c=C),
                            op=AO.add, axis=mybir.AxisListType.X)
                        cb += nt * kb[b]
                    colb += W * nt
                    idxb += icols

                nc.vector.tensor_tensor(out=agg_t[:], in0=aggb[0][:],
                                        in1=aggb[1][:], op=AO.add)
                nc.vector.tensor_tensor(out=agg_t[:], in0=agg_t[:],
                                        in1=aggb[2][:], op=AO.add)
                nc.vector.tensor_tensor(out=agg_t[:], in0=agg_t[:],
                                        in1=aggb[3][:], op=AO.add)
                # epilogue; d2 holds (1-ALPHA)*degree^2
                nc.vector.tensor_tensor(
                    out=g3(g_t), in0=g3(agg_t), in1=bcast_last(d2_t[:], C),
                    op=AO.mult)
                nc.vector.scalar_tensor_tensor(
                    out=g_t[:], in0=h_t[:], scalar=ALPHA, in1=g_t[:],
                    op0=AO.mult, op1=AO.add)
                nc.vector.tensor_reduce(
                    out=rs_t[:], in_=g3(g_t), op=AO.add,
                    axis=mybir.AxisListType.X)
                nc.vector.reciprocal(out=rs_t[:], in_=rs_t[:])
                nc.vector.tensor_tensor(
                    out=g3(h_t), in0=g3(g_t), in1=bcast_last(rs_t[:], C),
                    op=AO.mult)

            nc.sync.dma_start(out=hout_d[:], in_=g3(h_t))

    nc.compile()
    return nc, chunks


def _prep(x, W, edge_weight, degree, edge_index):
    src = edge_index[0].astype(np.int64)
    dst = edge_index[1].astype(np.int64)
    indeg = np.bincount(dst, minlength=N)

    pos_of = np.empty(N, dtype=np.int64)
    core_of = np.empty(N, dtype=np.int64)
    for cc in range(N_CORES):
        ids = np.arange(cc * 12500, (cc + 1) * 12500)
        order = ids[np.argsort(-indeg[ids], kind="stable")]
        core_of[order] = cc
        pos_of[order] = np.arange(12500)
    tile_of = pos_of // P
    part_of = pos_of % P
    row_of = core_of * PER_CORE + part_of * TILES + tile_of

    bank_of_row = row_of % NBANK
    loc_of_row = row_of // NBANK

    # per (core, part, tile, bank) counts -> K_tb = max over cores+parts
    eb = bank_of_row[src]
    key = ((core_of[dst] * P + part_of[dst]) * TILES + tile_of[dst]) * NBANK + eb
    cnt = np.bincount(key, minlength=N_CORES * P * TILES * NBANK)
    cnt = cnt.reshape(N_CORES, P, TILES, NBANK)
    K_tb = cnt.max(axis=(0, 1))                      # [TILES, NBANK]

    chunks = _plan(K_tb)
    # column base for (t, b): columns laid chunk-major, bank-major inside
    col_of = np.zeros((TILES, NBANK), dtype=np.int64)
    kb_of = np.zeros((TILES, NBANK), dtype=np.int64)
    colb = 0
    for (t0, nt, kb) in chunks:
        cb = colb
        for b in range(NBANK):
            for t in range(t0, t0 + nt):
                col_of[t, b] = cb + (t - t0) * kb[b]
                kb_of[t, b] = kb[b]
            cb += nt * kb[b]
        colb += nt * sum(kb)
    total_cols = colb

    # slot position for each edge: within-group rank
    order = np.argsort(key, kind="stable")
    ks = key[order]
    start = np.zeros(key.max() + 2, dtype=np.int64)
    cnts = np.bincount(ks)
    start[1:len(cnts) + 1] = np.cumsum(cnts)
    kslot = np.arange(len(ks)) - start[ks]

    ec, ep = core_of[dst[order]], part_of[dst[order]]
    et, ebk = tile_of[dst[order]], eb[order]
    col = col_of[et, ebk] + kslot

    ew_h = np.zeros((N_CORES, P, total_cols), dtype=np.float32)
    iv_h = np.zeros((N_CORES, P, total_cols), dtype=np.int16)
    ew_h[ec, ep, col] = edge_weight[order]
    iv_h[ec, ep, col] = loc_of_row[src[order]].astype(np.int16)

    # wrapped int16 idx tensors, concatenated per chunk/bank region
    gi_list = []
    for (t0, nt, kb) in chunks:
        for b in range(NBANK):
            if kb[b] == 0:
                continue
            c0 = col_of[t0, b]
            ncols = nt * kb[b]
            vals = iv_h[:, :, c0:c0 + ncols]             # [NC, P, ncols]
            # position i = col*128 + p ; wrapped [q, m] = value(m*16 + q%16)
            flat = vals.transpose(0, 2, 1).reshape(N_CORES, ncols * 128)
            wr = flat.reshape(N_CORES, ncols * 8, 16).transpose(0, 2, 1)
            wr = np.tile(wr, (1, 8, 1))                  # replicate to 128
            gi_list.append(wr)
    gi = np.concatenate(gi_list, axis=2)

    h0 = np.ones((N_CORES, P, TILES, C), dtype=np.float32)
    wv = np.zeros((N_CORES, P, TILES), dtype=np.float32)
    d2 = np.zeros((N_CORES, P, TILES), dtype=np.float32)
    h0[core_of, part_of, tile_of] = np.asarray(x)
    wv[core_of, part_of, tile_of] = np.asarray(W).reshape(-1)
    d2[core_of, part_of, tile_of] = (1.0 - ALPHA) * np.asarray(degree) ** 2
    return (K_tb, gi, ew_h, h0, wv, d2, core_of, part_of, tile_of,
            chunks, col_of, kb_of)


def kernel(x, W, edge_weight, degree, edge_index):
    global _compiled
    from concourse import bass_utils
    bass_utils.upload_artifacts = lambda tmpdir: "local://" + tmpdir

    pr = _prep(np.asarray(x), np.asarray(W), np.asarray(edge_weight),
               np.asarray(degree), np.asarray(edge_index))
    K_tb, gi, ew, h0, wv, d2, core_of, part_of, tile_of = pr[:9]

    key = tuple(int(v) for v in K_tb.ravel())
    if _compiled is None or _compiled[0] != key:
        _compiled = (key, _build(K_tb)[0])
    nc = _compiled[1]

    in_maps = [{"h0": h0[cc], "w": wv[cc], "d2": d2[cc],
                "gi": gi[cc], "ew": ew[cc]} for cc in range(N_CORES)]
    res = bass_utils.run_bass_kernel_spmd(
        nc, in_maps, core_ids=list(range(N_CORES)))
    houts = np.stack([res.results[cc]["hout"] for cc in range(N_CORES)])
    return houts[core_of, part_of, tile_of]


# revision 4
# speedup vs baseline: 1.1714x; 1.1601x over previous
"""GNN message-passing kernel for Trainium2 (8 NeuronCores).

Gathers use InstDMAGatherAnt (vectorized Q7 descriptor generation, one
instruction per chunk x bank) from a 256B-strided replicated g-table in
DRAM, instead of one indirect DMA per 128-row round (the v1 baseline pays
~1us of serialized SWDGE setup per round).  Table rows are banked
r -> (r % 4, r // 4) so local indices fit int16 (stride 1024B, base
offset bank*256B).  Slots per tile are padded per-bank to the
max-over-cores/partitions count so reduces stay rectangular; pad slots
gather a real row but carry edge weight 0.
"""
import sys
sys.path.insert(0, '/opt/trn_rl_repo')
import numpy as np

N = 100000
C = 16
LAYERS = 4
ALPHA = 0.5
N_CORES = 8
P = 128
PER_CORE = 12544            # 98 tiles of 128 (12500 real + 44 pad)
TILES = PER_CORE // P       # 98
NT = N_CORES * PER_CORE
NBANK = 4
NB_ROWS = NT // NBANK       # 25088 rows per bank (< 32768, int16-safe)
ROWW_E = 128                # strided table row width in bf16 elems (256B)
S_CH = 448                  # max msgs columns per chunk

_compiled = None


def _plan(K_tb):
    """K_tb: [TILES, NBANK] per-tile per-bank slot counts.
    Chunks of consecutive tiles with per-chunk-uniform bank widths
    Kb = max over chunk tiles. Returns list of
    (t0, nt, [K0..K3]) with nt * sum(Kb) <= S_CH."""
    chunks = []
    t = 0
    while t < TILES:
        nt = 1
        while t + nt <= TILES - 1 + 1:
            if t + nt > TILES - 1:
                break
            kb = K_tb[t:t + nt + 1].max(axis=0)
            if (nt + 1) * int(kb.sum()) > S_CH:
                break
            nt += 1
        kb = K_tb[t:t + nt].max(axis=0)
        chunks.append((t, nt, [int(v) for v in kb]))
        t += nt
    return chunks


def _dma_gather(eng, out_ap, in_ap, idxs_ap, num_idxs, elem_size, elem_step):
    """bass.dma_gather minus the 256B *payload* assert (the real HW
    constraint is on the row stride, which stays 256B-aligned here)."""
    import concourse.mybir as mybir
    from concourse import ap_utils
    assert idxs_ap.dtype == mybir.dt.int16
    assert in_ap.dtype == out_ap.dtype
    stride_bytes = elem_step * mybir.dt.size(in_ap.dtype)
    assert stride_bytes % 256 == 0 and stride_bytes // 256 < 256
    assert in_ap.ap[0][0] == elem_step
    assert ap_utils.ap_is_contiguous(in_ap.ap[1:])
    assert ap_utils.ap_is_contiguous(out_ap.ap[1:])
    assert ap_utils.ap_is_contiguous(idxs_ap.ap[1:])
    assert in_ap.ap[-1][1] == elem_size and out_ap.ap[-1][1] == elem_size
    assert out_ap.ap[0][1] * out_ap.ap[1][1] == num_idxs and num_idxs % 128 == 0
    _in_ap = eng.lower_ap_dma(in_ap, for_custom_bir_dma=True)
    _idxs_ap = eng.lower_ap(idxs_ap)
    _out_ap = eng.lower_ap(out_ap)
    return eng.add_instruction(mybir.InstDMAGatherAnt(
        name=eng.bass.get_next_instruction_name(),
        ins=[*_in_ap, _idxs_ap, eng.lower_val_access(eng.to_reg(num_idxs))],
        outs=[_out_ap], transpose=False,
        num_idxs=num_idxs, elem_size=elem_size,
        stride_bytes_256=stride_bytes // 256, gen_mode=0,
        single_packet=False, queue_num=0,
        sbuf_tokens_per_rank=0, sbuf_free_dim_per_rank=0,
        sbuf_free_dim_pad_per_rank=0, sbuf_byte_offset=0))


def _build(K_tb):
    import concourse.bass as bass
    import concourse.bacc as bacc
    import concourse.mybir as mybir
    from concourse.tile import TileContext

    AO = mybir.AluOpType
    f32 = mybir.dt.float32
    bf16 = mybir.dt.bfloat16
    K_tb = np.asarray(K_tb)
    chunks = _plan(K_tb)
    total_cols = sum(nt * sum(kb) for (_, nt, kb) in chunks)
    idx_cols = [128 * nt * sum(kb) // 16 for (_, nt, kb) in chunks]
    max_cols = max(nt * sum(kb) for (_, nt, kb) in chunks)
    max_idx_cols = max(idx_cols)

    nc = bacc.Bacc("TRN2", target_bir_lowering=False, debug=False,
                   num_devices=N_CORES)

    h0_d = nc.dram_tensor("h0", [P, TILES, C], f32, kind="ExternalInput")
    w_d = nc.dram_tensor("w", [P, TILES], f32, kind="ExternalInput")
    d2_d = nc.dram_tensor("d2", [P, TILES], f32, kind="ExternalInput")
    gi_d = nc.dram_tensor("gi", [P, sum(idx_cols)], mybir.dt.int16,
                          kind="ExternalInput")
    ew_d = nc.dram_tensor("ew", [P, total_cols], bf16, kind="ExternalInput")
    hout_d = nc.dram_tensor("hout", [P, TILES, C], f32,
                            kind="ExternalOutput")

    gslice_d = nc.dram_tensor("gslice", [PER_CORE, C], bf16)
    gfull_c = nc.dram_tensor("gfullc", [NT, C], bf16, addr_space="Shared")
    gfull_s = nc.dram_tensor("gfulls", [NT, ROWW_E], bf16)
    rg = [list(range(N_CORES))]

    def bcast_last(ap, c):
        return bass.AP(ap.tensor, ap.offset, [*ap.ap, [0, c]])

    def g3(t):
        return t[:].rearrange("p (t c) -> p t c", c=C)

    with TileContext(nc) as tc:
        with tc.tile_pool(name="const", bufs=1) as cpool, \
             tc.tile_pool(name="gpool", bufs=3) as gpool:
            ew_t = cpool.tile([P, total_cols], bf16)
            nc.sync.dma_start(out=ew_t[:], in_=ew_d[:])
            w_t = cpool.tile([P, TILES], f32)
            nc.sync.dma_start(out=w_t[:], in_=w_d[:])
            d2_t = cpool.tile([P, TILES], f32)
            nc.sync.dma_start(out=d2_t[:], in_=d2_d[:])
            h_t = cpool.tile([P, TILES * C], f32)
            nc.sync.dma_start(out=g3(h_t), in_=h0_d[:])
            g_t = cpool.tile([P, TILES * C], f32)
            gb_t = cpool.tile([P, TILES * C], bf16)
            agg_t = cpool.tile([P, TILES * C], f32)
            aggb = [cpool.tile([P, TILES * C], f32, name="aggb%d" % b)
                    for b in range(NBANK)]
            rs_t = cpool.tile([P, TILES], f32)

            def exchange():
                # push local g to every core's strided gather table:
                # AllGather (compact) then expand to 256B-strided rows,
                # one copy per bank (rows r = 4l+b) so bank-b gathers
                # only wait on their own expand; per-bank copies also
                # keep AP dims under the 16-bit limit
                nc.gpsimd.collective_compute(
                    "AllGather", AO.bypass,
                    ins=[gslice_d[:]], outs=[gfull_c[:]],
                    replica_groups=rg)
                for b in range(NBANK):
                    nc.sync.dma_start(
                        out=bass.AP(gfull_s[:].tensor, b * ROWW_E,
                                    [[NBANK * ROWW_E, NB_ROWS], [1, C]]),
                        in_=bass.AP(gfull_c[:].tensor, b * C,
                                    [[NBANK * C, NB_ROWS], [1, C]]))

            gsl = gslice_d[:].rearrange("(p x) c -> p (x c)", p=P)
            nc.vector.tensor_tensor(
                out=g3(gb_t), in0=g3(h_t), in1=bcast_last(w_t[:], C),
                op=AO.mult)
            nc.sync.dma_start(out=gsl, in_=gb_t[:])
            exchange()

            for layer in range(LAYERS):
                colb = 0
                idxb = 0
                for ci, (t0, nt, kb) in enumerate(chunks):
                    W = sum(kb)
                    icols = idx_cols[ci]
                    idx_t = gpool.tile([P, max_idx_cols], mybir.dt.int16,
                                       tag="idx")
                    nc.sync.dma_start(out=idx_t[:, :icols],
                                      in_=gi_d[:, idxb:idxb + icols])
                    msgs = gpool.tile([P, max_cols * C], bf16, tag="msgs")
                    cb = 0
                    ib = 0
                    for b in range(NBANK):
                        if kb[b] == 0:
                            continue
                        ncols = nt * kb[b]
                        nidx = 128 * ncols
                        src = bass.AP(gfull_s[:].tensor, b * ROWW_E,
                                      [[NBANK * ROWW_E, NB_ROWS], [1, C]])
                        _dma_gather(
                            nc.gpsimd,
                            out_ap=msgs[:, cb * C:(cb + ncols) * C]
                                .rearrange("p (s c) -> p s c", c=C),
                            in_ap=src,
                            idxs_ap=idx_t[:, ib:ib + nidx // 16],
                            num_idxs=nidx, elem_size=C,
                            elem_step=NBANK * ROWW_E)
                        cb += ncols
                        ib += nidx // 16
                    nc.vector.tensor_tensor(
                        out=msgs[:, :W * nt * C].rearrange(
                            "p (s c) -> p s c", c=C),
                        in0=msgs[:, :W * nt * C].rearrange(
                            "p (s c) -> p s c", c=C),
                        in1=bcast_last(ew_t[:, colb:colb + W * nt], C),
                        op=AO.mult)
                    cb = 0
                    for b in range(NBANK):
                        if kb[b] == 0:
                            continue
                        nc.vector.tensor_reduce(
                            out=aggb[b][:, t0 * C:(t0 + nt) * C],
                            in_=msgs[:, cb * C:(cb + nt * kb[b]) * C]
                                .rearrange("p (t k c) -> p t c k",
                                           k=kb[b], c=C),
                            op=AO.add, axis=mybir.AxisListType.X)
                        cb += nt * kb[b]
                    colb += W * nt
                    idxb += icols

                    # per-chunk epilogue for tiles [t0, t0+nt): the
                    # h-update, next layer's g, and its gslice range all
                    # land while later chunks are still gathering, so
                    # only AllGather+expand stay serial between layers
                    sl = slice(t0 * C, (t0 + nt) * C)
                    ts = slice(t0, t0 + nt)

                    def s3(t):
                        return t[:, sl].rearrange("p (t c) -> p t c", c=C)

                    nc.vector.tensor_tensor(out=agg_t[:, sl],
                                            in0=aggb[0][:, sl],
                                            in1=aggb[1][:, sl], op=AO.add)
                    nc.vector.tensor_tensor(out=agg_t[:, sl],
                                            in0=agg_t[:, sl],
                                            in1=aggb[2][:, sl], op=AO.add)
                    nc.vector.tensor_tensor(out=agg_t[:, sl],
                                            in0=agg_t[:, sl],
                                            in1=aggb[3][:, sl], op=AO.add)
                    # d2 holds (1-ALPHA)*degree^2
                    nc.vector.tensor_tensor(
                        out=s3(g_t), in0=s3(agg_t),
                        in1=bcast_last(d2_t[:, ts], C), op=AO.mult)
                    nc.vector.scalar_tensor_tensor(
                        out=g_t[:, sl], in0=h_t[:, sl], scalar=ALPHA,
                        in1=g_t[:, sl], op0=AO.mult, op1=AO.add)
                    nc.vector.tensor_reduce(
                        out=rs_t[:, ts], in_=s3(g_t), op=AO.add,
                        axis=mybir.AxisListType.X)
                    nc.vector.reciprocal(out=rs_t[:, ts], in_=rs_t[:, ts])
                    nc.vector.tensor_tensor(
                        out=s3(h_t), in0=s3(g_t),
                        in1=bcast_last(rs_t[:, ts], C), op=AO.mult)
                    if layer < LAYERS - 1:
                        nc.vector.tensor_tensor(
                            out=gb_t[:, sl].rearrange(
                                "p (t c) -> p t c", c=C),
                            in0=s3(h_t),
                            in1=bcast_last(w_t[:, ts], C), op=AO.mult)
                        nc.sync.dma_start(out=gsl[:, sl], in_=gb_t[:, sl])

                if layer < LAYERS - 1:
                    exchange()

            nc.sync.dma_start(out=hout_d[:], in_=g3(h_t))

    nc.compile()
    return nc, chunks


def _prep(x, W, edge_weight, degree, edge_index):
    src = edge_index[0].astype(np.int64)
    dst = edge_index[1].astype(np.int64)
    indeg = np.bincount(dst, minlength=N)

    pos_of = np.empty(N, dtype=np.int64)
    core_of = np.empty(N, dtype=np.int64)
    for cc in range(N_CORES):
        ids = np.arange(cc * 12500, (cc + 1) * 12500)
        order = ids[np.argsort(-indeg[ids], kind="stable")]
        core_of[order] = cc
        pos_of[order] = np.arange(12500)
    tile_of = pos_of // P
    part_of = pos_of % P
    row_of = core_of * PER_CORE + part_of * TILES + tile_of

    bank_of_row = row_of % NBANK
    loc_of_row = row_of // NBANK

    # per (core, part, tile, bank) counts -> K_tb = max over cores+parts
    eb = bank_of_row[src]
    key = ((core_of[dst] * P + part_of[dst]) * TILES + tile_of[dst]) * NBANK + eb
    cnt = np.bincount(key, minlength=N_CORES * P * TILES * NBANK)
    cnt = cnt.reshape(N_CORES, P, TILES, NBANK)
    K_tb = cnt.max(axis=(0, 1))                      # [TILES, NBANK]

    chunks = _plan(K_tb)
    # column base for (t, b): columns laid chunk-major, bank-major inside
    col_of = np.zeros((TILES, NBANK), dtype=np.int64)
    kb_of = np.zeros((TILES, NBANK), dtype=np.int64)
    colb = 0
    for (t0, nt, kb) in chunks:
        cb = colb
        for b in range(NBANK):
            for t in range(t0, t0 + nt):
                col_of[t, b] = cb + (t - t0) * kb[b]
                kb_of[t, b] = kb[b]
            cb += nt * kb[b]
        colb += nt * sum(kb)
    total_cols = colb

    # slot position for each edge: within-group rank
    order = np.argsort(key, kind="stable")
    ks = key[order]
    start = np.zeros(key.max() + 2, dtype=np.int64)
    cnts = np.bincount(ks)
    start[1:len(cnts) + 1] = np.cumsum(cnts)
    kslot = np.arange(len(ks)) - start[ks]

    ec, ep = core_of[dst[order]], part_of[dst[order]]
    et, ebk = tile_of[dst[order]], eb[order]
    col = col_of[et, ebk] + kslot

    import ml_dtypes
    ew_h = np.zeros((N_CORES, P, total_cols), dtype=ml_dtypes.bfloat16)
    iv_h = np.zeros((N_CORES, P, total_cols), dtype=np.int16)
    ew_h[ec, ep, col] = edge_weight[order].astype(ml_dtypes.bfloat16)
    iv_h[ec, ep, col] = loc_of_row[src[order]].astype(np.int16)

    # wrapped int16 idx tensors, concatenated per chunk/bank region
    gi_list = []
    for (t0, nt, kb) in chunks:
        for b in range(NBANK):
            if kb[b] == 0:
                continue
            c0 = col_of[t0, b]
            ncols = nt * kb[b]
            vals = iv_h[:, :, c0:c0 + ncols]             # [NC, P, ncols]
            # position i = col*128 + p ; wrapped [q, m] = value(m*16 + q%16)
            flat = vals.transpose(0, 2, 1).reshape(N_CORES, ncols * 128)
            wr = flat.reshape(N_CORES, ncols * 8, 16).transpose(0, 2, 1)
            wr = np.tile(wr, (1, 8, 1))                  # replicate to 128
            gi_list.append(wr)
    gi = np.concatenate(gi_list, axis=2)

    h0 = np.ones((N_CORES, P, TILES, C), dtype=np.float32)
    wv = np.zeros((N_CORES, P, TILES), dtype=np.float32)
    d2 = np.zeros((N_CORES, P, TILES), dtype=np.float32)
    h0[core_of, part_of, tile_of] = np.asarray(x)
    wv[core_of, part_of, tile_of] = np.asarray(W).reshape(-1)
    d2[core_of, part_of, tile_of] = (1.0 - ALPHA) * np.asarray(degree) ** 2
    return (K_tb, gi, ew_h, h0, wv, d2, core_of, part_of, tile_of,
            chunks, col_of, kb_of)


def kernel(x, W, edge_weight, degree, edge_index):
    global _compiled
    from concourse import bass_utils
    bass_utils.upload_artifacts = lambda tmpdir: "local://" + tmpdir

    pr = _prep(np.asarray(x), np.asarray(W), np.asarray(edge_weight),
               np.asarray(degree), np.asarray(edge_index))
    K_tb, gi, ew, h0, wv, d2, core_of, part_of, tile_of = pr[:9]

    key = tuple(int(v) for v in K_tb.ravel())
    if _compiled is None or _compiled[0] != key:
        _compiled = (key, _build(K_tb)[0])
    nc = _compiled[1]

    in_maps = [{"h0": h0[cc], "w": wv[cc], "d2": d2[cc],
                "gi": gi[cc], "ew": ew[cc]} for cc in range(N_CORES)]
    res = bass_utils.run_bass_kernel_spmd(
        nc, in_maps, core_ids=list(range(N_CORES)))
    houts = np.stack([res.results[cc]["hout"] for cc in range(N_CORES)])
    return houts[core_of, part_of, tile_of]


# revision 5
# speedup vs baseline: 1.1934x; 1.0187x over previous
"""GNN message-passing kernel for Trainium2 (8 NeuronCores).

Gathers use InstDMAGatherAnt (vectorized Q7 descriptor generation, one
instruction per chunk x bank) from a 256B-strided replicated g-table in
DRAM, instead of one indirect DMA per 128-row round (the v1 baseline pays
~1us of serialized SWDGE setup per round).  Table rows are banked
r -> (r % 4, r // 4) so local indices fit int16 (stride 1024B, base
offset bank*256B).  Slots per tile are padded per-bank to the
max-over-cores/partitions count so reduces stay rectangular; pad slots
gather a real row but carry edge weight 0.
"""
import sys
sys.path.insert(0, '/opt/trn_rl_repo')
import numpy as np

N = 100000
C = 16
LAYERS = 4
ALPHA = 0.5
N_CORES = 8
P = 128
PER_CORE = 12544            # 98 tiles of 128 (12500 real + 44 pad)
TILES = PER_CORE // P       # 98
NT = N_CORES * PER_CORE
NBANK = 4
NB_ROWS = NT // NBANK       # 25088 rows per bank (< 32768, int16-safe)
ROWW_E = 128                # strided table row width in bf16 elems (256B)
S_CH = 256                  # max msgs columns per chunk

_compiled = None


def _plan(K_tb):
    """K_tb: [TILES, NBANK] per-tile per-bank slot counts.
    Chunks of consecutive tiles with per-chunk-uniform bank widths
    Kb = max over chunk tiles. Returns list of
    (t0, nt, [K0..K3]) with nt * sum(Kb) <= S_CH."""
    chunks = []
    t = 0
    while t < TILES:
        nt = 1
        while t + nt <= TILES - 1 + 1:
            if t + nt > TILES - 1:
                break
            kb = K_tb[t:t + nt + 1].max(axis=0)
            if (nt + 1) * int(kb.sum()) > S_CH:
                break
            nt += 1
        kb = K_tb[t:t + nt].max(axis=0)
        chunks.append((t, nt, [int(v) for v in kb]))
        t += nt
    return chunks


def _dma_gather(eng, out_ap, in_ap, idxs_ap, num_idxs, elem_size, elem_step):
    """bass.dma_gather minus the 256B *payload* assert (the real HW
    constraint is on the row stride, which stays 256B-aligned here)."""
    import concourse.mybir as mybir
    from concourse import ap_utils
    assert idxs_ap.dtype == mybir.dt.int16
    assert in_ap.dtype == out_ap.dtype
    stride_bytes = elem_step * mybir.dt.size(in_ap.dtype)
    assert stride_bytes % 256 == 0 and stride_bytes // 256 < 256
    assert in_ap.ap[0][0] == elem_step
    assert ap_utils.ap_is_contiguous(in_ap.ap[1:])
    assert ap_utils.ap_is_contiguous(out_ap.ap[1:])
    assert ap_utils.ap_is_contiguous(idxs_ap.ap[1:])
    assert in_ap.ap[-1][1] == elem_size and out_ap.ap[-1][1] == elem_size
    assert out_ap.ap[0][1] * out_ap.ap[1][1] == num_idxs and num_idxs % 128 == 0
    _in_ap = eng.lower_ap_dma(in_ap, for_custom_bir_dma=True)
    _idxs_ap = eng.lower_ap(idxs_ap)
    _out_ap = eng.lower_ap(out_ap)
    return eng.add_instruction(mybir.InstDMAGatherAnt(
        name=eng.bass.get_next_instruction_name(),
        ins=[*_in_ap, _idxs_ap, eng.lower_val_access(eng.to_reg(num_idxs))],
        outs=[_out_ap], transpose=False,
        num_idxs=num_idxs, elem_size=elem_size,
        stride_bytes_256=stride_bytes // 256, gen_mode=0,
        single_packet=False, queue_num=0,
        sbuf_tokens_per_rank=0, sbuf_free_dim_per_rank=0,
        sbuf_free_dim_pad_per_rank=0, sbuf_byte_offset=0))


def _build(K_tb):
    import concourse.bass as bass
    import concourse.bacc as bacc
    import concourse.mybir as mybir
    from concourse.tile import TileContext

    AO = mybir.AluOpType
    f32 = mybir.dt.float32
    bf16 = mybir.dt.bfloat16
    K_tb = np.asarray(K_tb)
    chunks = _plan(K_tb)
    total_cols = sum(nt * sum(kb) for (_, nt, kb) in chunks)
    idx_cols = [128 * nt * sum(kb) // 16 for (_, nt, kb) in chunks]
    max_cols = max(nt * sum(kb) for (_, nt, kb) in chunks)
    max_idx_cols = max(idx_cols)

    nc = bacc.Bacc("TRN2", target_bir_lowering=False, debug=False,
                   num_devices=N_CORES)

    h0_d = nc.dram_tensor("h0", [P, TILES, C], f32, kind="ExternalInput")
    w_d = nc.dram_tensor("w", [P, TILES], f32, kind="ExternalInput")
    d2_d = nc.dram_tensor("d2", [P, TILES], f32, kind="ExternalInput")
    gi_d = nc.dram_tensor("gi", [P, sum(idx_cols)], mybir.dt.int16,
                          kind="ExternalInput")
    ew_d = nc.dram_tensor("ew", [P, total_cols], bf16, kind="ExternalInput")
    hout_d = nc.dram_tensor("hout", [P, TILES, C], f32,
                            kind="ExternalOutput")

    gslice_d = nc.dram_tensor("gslice", [PER_CORE, C], bf16)
    gfull_c = nc.dram_tensor("gfullc", [NT, C], bf16, addr_space="Shared")
    gfull_s = nc.dram_tensor("gfulls", [NT, ROWW_E], bf16)
    rg = [list(range(N_CORES))]

    def bcast_last(ap, c):
        return bass.AP(ap.tensor, ap.offset, [*ap.ap, [0, c]])

    def g3(t):
        return t[:].rearrange("p (t c) -> p t c", c=C)

    with TileContext(nc) as tc:
        with tc.tile_pool(name="const", bufs=1) as cpool, \
             tc.tile_pool(name="gpool", bufs=3) as gpool:
            ew_t = cpool.tile([P, total_cols], bf16)
            nc.sync.dma_start(out=ew_t[:], in_=ew_d[:])
            w_t = cpool.tile([P, TILES], f32)
            nc.sync.dma_start(out=w_t[:], in_=w_d[:])
            d2_t = cpool.tile([P, TILES], f32)
            nc.sync.dma_start(out=d2_t[:], in_=d2_d[:])
            h_t = cpool.tile([P, TILES * C], f32)
            nc.sync.dma_start(out=g3(h_t), in_=h0_d[:])
            g_t = cpool.tile([P, TILES * C], f32)
            gb_t = cpool.tile([P, TILES * C], bf16)
            agg_t = cpool.tile([P, TILES * C], f32)
            aggb = [cpool.tile([P, TILES * C], f32, name="aggb%d" % b)
                    for b in range(NBANK)]
            rs_t = cpool.tile([P, TILES], f32)

            def exchange():
                # push local g to every core's strided gather table:
                # AllGather (compact) then expand to 256B-strided rows,
                # one copy per bank (rows r = 4l+b) so bank-b gathers
                # only wait on their own expand; per-bank copies also
                # keep AP dims under the 16-bit limit
                nc.gpsimd.collective_compute(
                    "AllGather", AO.bypass,
                    ins=[gslice_d[:]], outs=[gfull_c[:]],
                    replica_groups=rg)
                for b in range(NBANK):
                    nc.sync.dma_start(
                        out=bass.AP(gfull_s[:].tensor, b * ROWW_E,
                                    [[NBANK * ROWW_E, NB_ROWS], [1, C]]),
                        in_=bass.AP(gfull_c[:].tensor, b * C,
                                    [[NBANK * C, NB_ROWS], [1, C]]))

            gsl = gslice_d[:].rearrange("(p x) c -> p (x c)", p=P)
            nc.vector.tensor_tensor(
                out=g3(gb_t), in0=g3(h_t), in1=bcast_last(w_t[:], C),
                op=AO.mult)
            nc.sync.dma_start(out=gsl, in_=gb_t[:])
            exchange()

            for layer in range(LAYERS):
                colb = 0
                idxb = 0
                for ci, (t0, nt, kb) in enumerate(chunks):
                    W = sum(kb)
                    icols = idx_cols[ci]
                    idx_t = gpool.tile([P, max_idx_cols], mybir.dt.int16,
                                       tag="idx")
                    nc.sync.dma_start(out=idx_t[:, :icols],
                                      in_=gi_d[:, idxb:idxb + icols])
                    msgs = gpool.tile([P, max_cols * C], bf16, tag="msgs")
                    cb = 0
                    ib = 0
                    for b in range(NBANK):
                        if kb[b] == 0:
                            continue
                        ncols = nt * kb[b]
                        nidx = 128 * ncols
                        src = bass.AP(gfull_s[:].tensor, b * ROWW_E,
                                      [[NBANK * ROWW_E, NB_ROWS], [1, C]])
                        _dma_gather(
                            nc.gpsimd,
                            out_ap=msgs[:, cb * C:(cb + ncols) * C]
                                .rearrange("p (s c) -> p s c", c=C),
                            in_ap=src,
                            idxs_ap=idx_t[:, ib:ib + nidx // 16],
                            num_idxs=nidx, elem_size=C,
                            elem_step=NBANK * ROWW_E)
                        cb += ncols
                        ib += nidx // 16
                    nc.vector.tensor_tensor(
                        out=msgs[:, :W * nt * C].rearrange(
                            "p (s c) -> p s c", c=C),
                        in0=msgs[:, :W * nt * C].rearrange(
                            "p (s c) -> p s c", c=C),
                        in1=bcast_last(ew_t[:, colb:colb + W * nt], C),
                        op=AO.mult)
                    cb = 0
                    for b in range(NBANK):
                        if kb[b] == 0:
                            continue
                        nc.vector.tensor_reduce(
                            out=aggb[b][:, t0 * C:(t0 + nt) * C],
                            in_=msgs[:, cb * C:(cb + nt * kb[b]) * C]
                                .rearrange("p (t k c) -> p t c k",
                                           k=kb[b], c=C),
                            op=AO.add, axis=mybir.AxisListType.X)
                        cb += nt * kb[b]
                    colb += W * nt
                    idxb += icols

                    # per-chunk epilogue for tiles [t0, t0+nt): the
                    # h-update, next layer's g, and its gslice range all
                    # land while later chunks are still gathering, so
                    # only AllGather+expand stay serial between layers
                    sl = slice(t0 * C, (t0 + nt) * C)
                    ts = slice(t0, t0 + nt)

                    def s3(t):
                        return t[:, sl].rearrange("p (t c) -> p t c", c=C)

                    nc.vector.tensor_tensor(out=agg_t[:, sl],
                                            in0=aggb[0][:, sl],
                                            in1=aggb[1][:, sl], op=AO.add)
                    nc.vector.tensor_tensor(out=agg_t[:, sl],
                                            in0=agg_t[:, sl],
                                            in1=aggb[2][:, sl], op=AO.add)
                    nc.vector.tensor_tensor(out=agg_t[:, sl],
                                            in0=agg_t[:, sl],
                                            in1=aggb[3][:, sl], op=AO.add)
                    # d2 holds (1-ALPHA)*degree^2
                    nc.vector.tensor_tensor(
                        out=s3(g_t), in0=s3(agg_t),
                        in1=bcast_last(d2_t[:, ts], C), op=AO.mult)
                    nc.vector.scalar_tensor_tensor(
                        out=g_t[:, sl], in0=h_t[:, sl], scalar=ALPHA,
                        in1=g_t[:, sl], op0=AO.mult, op1=AO.add)
                    nc.vector.tensor_reduce(
                        out=rs_t[:, ts], in_=s3(g_t), op=AO.add,
                        axis=mybir.AxisListType.X)
                    nc.vector.reciprocal(out=rs_t[:, ts], in_=rs_t[:, ts])
                    nc.vector.tensor_tensor(
                        out=s3(h_t), in0=s3(g_t),
                        in1=bcast_last(rs_t[:, ts], C), op=AO.mult)
                    if layer < LAYERS - 1:
                        nc.vector.tensor_tensor(
                            out=gb_t[:, sl].rearrange(
                                "p (t c) -> p t c", c=C),
                            in0=s3(h_t),
                            in1=bcast_last(w_t[:, ts], C), op=AO.mult)
                        nc.sync.dma_start(out=gsl[:, sl], in_=gb_t[:, sl])

                if layer < LAYERS - 1:
                    exchange()

            nc.sync.dma_start(out=hout_d[:], in_=g3(h_t))

    nc.compile()
    return nc, chunks


def _prep(x, W, edge_weight, degree, edge_index):
    src = edge_index[0].astype(np.int64)
    dst = edge_index[1].astype(np.int64)
    indeg = np.bincount(dst, minlength=N)

    pos_of = np.empty(N, dtype=np.int64)
    core_of = np.empty(N, dtype=np.int64)
    for cc in range(N_CORES):
        ids = np.arange(cc * 12500, (cc + 1) * 12500)
        order = ids[np.argsort(-indeg[ids], kind="stable")]
        core_of[order] = cc
        pos_of[order] = np.arange(12500)
    tile_of = pos_of // P
    part_of = pos_of % P
    row_of = core_of * PER_CORE + part_of * TILES + tile_of

    bank_of_row = row_of % NBANK
    loc_of_row = row_of // NBANK

    # per (core, part, tile, bank) counts -> K_tb = max over cores+parts
    eb = bank_of_row[src]
    key = ((core_of[dst] * P + part_of[dst]) * TILES + tile_of[dst]) * NBANK + eb
    cnt = np.bincount(key, minlength=N_CORES * P * TILES * NBANK)
    cnt = cnt.reshape(N_CORES, P, TILES, NBANK)
    K_tb = cnt.max(axis=(0, 1))                      # [TILES, NBANK]

    chunks = _plan(K_tb)
    # column base for (t, b): columns laid chunk-major, bank-major inside
    col_of = np.zeros((TILES, NBANK), dtype=np.int64)
    kb_of = np.zeros((TILES, NBANK), dtype=np.int64)
    colb = 0
    for (t0, nt, kb) in chunks:
        cb = colb
        for b in range(NBANK):
            for t in range(t0, t0 + nt):
                col_of[t, b] = cb + (t - t0) * kb[b]
                kb_of[t, b] = kb[b]
            cb += nt * kb[b]
        colb += nt * sum(kb)
    total_cols = colb

    # slot position for each edge: within-group rank
    order = np.argsort(key, kind="stable")
    ks = key[order]
    start = np.zeros(key.max() + 2, dtype=np.int64)
    cnts = np.bincount(ks)
    start[1:len(cnts) + 1] = np.cumsum(cnts)
    kslot = np.arange(len(ks)) - start[ks]

    ec, ep = core_of[dst[order]], part_of[dst[order]]
    et, ebk = tile_of[dst[order]], eb[order]
    col = col_of[et, ebk] + kslot

    import ml_dtypes
    ew_h = np.zeros((N_CORES, P, total_cols), dtype=ml_dtypes.bfloat16)
    iv_h = np.zeros((N_CORES, P, total_cols), dtype=np.int16)
    ew_h[ec, ep, col] = edge_weight[order].astype(ml_dtypes.bfloat16)
    iv_h[ec, ep, col] = loc_of_row[src[order]].astype(np.int16)

    # wrapped int16 idx tensors, concatenated per chunk/bank region
    gi_list = []
    for (t0, nt, kb) in chunks:
        for b in range(NBANK):
            if kb[b] == 0:
                continue
            c0 = col_of[t0, b]
            ncols = nt * kb[b]
            vals = iv_h[:, :, c0:c0 + ncols]             # [NC, P, ncols]
            # position i = col*128 + p ; wrapped [q, m] = value(m*16 + q%16)
            flat = vals.transpose(0, 2, 1).reshape(N_CORES, ncols * 128)
            wr = flat.reshape(N_CORES, ncols * 8, 16).transpose(0, 2, 1)
            wr = np.tile(wr, (1, 8, 1))                  # replicate to 128
            gi_list.append(wr)
    gi = np.concatenate(gi_list, axis=2)

    h0 = np.ones((N_CORES, P, TILES, C), dtype=np.float32)
    wv = np.zeros((N_CORES, P, TILES), dtype=np.float32)
    d2 = np.zeros((N_CORES, P, TILES), dtype=np.float32)
    h0[core_of, part_of, tile_of] = np.asarray(x)
    wv[core_of, part_of, tile_of] = np.asarray(W).reshape(-1)
    d2[core_of, part_of, tile_of] = (1.0 - ALPHA) * np.asarray(degree) ** 2
    return (K_tb, gi, ew_h, h0, wv, d2, core_of, part_of, tile_of,
            chunks, col_of, kb_of)


def kernel(x, W, edge_weight, degree, edge_index):
    global _compiled
    from concourse import bass_utils
    bass_utils.upload_artifacts = lambda tmpdir: "local://" + tmpdir

    pr = _prep(np.asarray(x), np.asarray(W), np.asarray(edge_weight),
               np.asarray(degree), np.asarray(edge_index))
    K_tb, gi, ew, h0, wv, d2, core_of, part_of, tile_of = pr[:9]

    key = tuple(int(v) for v in K_tb.ravel())
    if _compiled is None or _compiled[0] != key:
        _compiled = (key, _build(K_tb)[0])
    nc = _compiled[1]

    in_maps = [{"h0": h0[cc], "w": wv[cc], "d2": d2[cc],
                "gi": gi[cc], "ew": ew[cc]} for cc in range(N_CORES)]
    res = bass_utils.run_bass_kernel_spmd(
        nc, in_maps, core_ids=list(range(N_CORES)))
    houts = np.stack([res.results[cc]["hout"] for cc in range(N_CORES)])
    return houts[core_of, part_of, tile_of]


# revision 6
# speedup vs baseline: 1.2063x; 1.0108x over previous
"""GNN message-passing kernel for Trainium2 (8 NeuronCores).

Gathers use InstDMAGatherAnt (vectorized Q7 descriptor generation, one
instruction per chunk x bank) from a 256B-strided replicated g-table in
DRAM, instead of one indirect DMA per 128-row round (the v1 baseline pays
~1us of serialized SWDGE setup per round).  Table rows are banked
r -> (r % 4, r // 4) so local indices fit int16 (stride 1024B, base
offset bank*256B).  Slots per tile are padded per-bank to the
max-over-cores/partitions count so reduces stay rectangular; pad slots
gather a real row but carry edge weight 0.
"""
import sys
sys.path.insert(0, '/opt/trn_rl_repo')
import numpy as np

N = 100000
C = 16
LAYERS = 4
ALPHA = 0.5
N_CORES = 8
P = 128
PER_CORE = 12544            # 98 tiles of 128 (12500 real + 44 pad)
TILES = PER_CORE // P       # 98
NT = N_CORES * PER_CORE
NBANK = 4
NB_ROWS = NT // NBANK       # 25088 rows per bank (< 32768, int16-safe)
ROWW_E = 128                # strided table row width in bf16 elems (256B)
S_CH = 256                  # max msgs columns per chunk

_compiled = None


def _plan(K_tb):
    """K_tb: [TILES, NBANK] per-tile per-bank slot counts.
    Chunks of consecutive tiles with per-chunk-uniform bank widths
    Kb = max over chunk tiles. Returns list of
    (t0, nt, [K0..K3]) with nt * sum(Kb) <= S_CH."""
    chunks = []
    t = 0
    while t < TILES:
        nt = 1
        while t + nt <= TILES - 1 + 1:
            if t + nt > TILES - 1:
                break
            kb = K_tb[t:t + nt + 1].max(axis=0)
            if (nt + 1) * int(kb.sum()) > S_CH:
                break
            nt += 1
        kb = K_tb[t:t + nt].max(axis=0)
        chunks.append((t, nt, [int(v) for v in kb]))
        t += nt
    return chunks


def _dma_gather(eng, out_ap, in_ap, idxs_ap, num_idxs, elem_size, elem_step):
    """bass.dma_gather minus the 256B *payload* assert (the real HW
    constraint is on the row stride, which stays 256B-aligned here)."""
    import concourse.mybir as mybir
    from concourse import ap_utils
    assert idxs_ap.dtype == mybir.dt.int16
    assert in_ap.dtype == out_ap.dtype
    stride_bytes = elem_step * mybir.dt.size(in_ap.dtype)
    assert stride_bytes % 256 == 0 and stride_bytes // 256 < 256
    assert in_ap.ap[0][0] == elem_step
    assert ap_utils.ap_is_contiguous(in_ap.ap[1:])
    assert ap_utils.ap_is_contiguous(out_ap.ap[1:])
    assert ap_utils.ap_is_contiguous(idxs_ap.ap[1:])
    assert in_ap.ap[-1][1] == elem_size and out_ap.ap[-1][1] == elem_size
    assert out_ap.ap[0][1] * out_ap.ap[1][1] == num_idxs and num_idxs % 128 == 0
    _in_ap = eng.lower_ap_dma(in_ap, for_custom_bir_dma=True)
    _idxs_ap = eng.lower_ap(idxs_ap)
    _out_ap = eng.lower_ap(out_ap)
    return eng.add_instruction(mybir.InstDMAGatherAnt(
        name=eng.bass.get_next_instruction_name(),
        ins=[*_in_ap, _idxs_ap, eng.lower_val_access(eng.to_reg(num_idxs))],
        outs=[_out_ap], transpose=False,
        num_idxs=num_idxs, elem_size=elem_size,
        stride_bytes_256=stride_bytes // 256, gen_mode=0,
        single_packet=False, queue_num=0,
        sbuf_tokens_per_rank=0, sbuf_free_dim_per_rank=0,
        sbuf_free_dim_pad_per_rank=0, sbuf_byte_offset=0))


def _build(K_tb):
    import concourse.bass as bass
    import concourse.bacc as bacc
    import concourse.mybir as mybir
    from concourse.tile import TileContext

    AO = mybir.AluOpType
    f32 = mybir.dt.float32
    bf16 = mybir.dt.bfloat16
    K_tb = np.asarray(K_tb)
    chunks = _plan(K_tb)
    total_cols = sum(nt * sum(kb) for (_, nt, kb) in chunks)
    idx_cols = [128 * nt * sum(kb) // 16 for (_, nt, kb) in chunks]
    max_cols = max(nt * sum(kb) for (_, nt, kb) in chunks)
    max_idx_cols = max(idx_cols)

    nc = bacc.Bacc("TRN2", target_bir_lowering=False, debug=False,
                   num_devices=N_CORES)

    h0_d = nc.dram_tensor("h0", [P, TILES, C], f32, kind="ExternalInput")
    w_d = nc.dram_tensor("w", [P, TILES], f32, kind="ExternalInput")
    d2_d = nc.dram_tensor("d2", [P, TILES], f32, kind="ExternalInput")
    gi_d = nc.dram_tensor("gi", [P, sum(idx_cols)], mybir.dt.int16,
                          kind="ExternalInput")
    ew_d = nc.dram_tensor("ew", [P, total_cols], bf16, kind="ExternalInput")
    hout_d = nc.dram_tensor("hout", [P, TILES, C], f32,
                            kind="ExternalOutput")

    gslice_d = nc.dram_tensor("gslice", [PER_CORE, C], bf16)
    gfull_c = nc.dram_tensor("gfullc", [NT, C], bf16, addr_space="Shared")
    gfull_s = nc.dram_tensor("gfulls", [NT, ROWW_E], bf16)
    rg = [list(range(N_CORES))]

    def bcast_last(ap, c):
        return bass.AP(ap.tensor, ap.offset, [*ap.ap, [0, c]])

    def g3(t):
        return t[:].rearrange("p (t c) -> p t c", c=C)

    with TileContext(nc) as tc:
        with tc.tile_pool(name="const", bufs=1) as cpool, \
             tc.tile_pool(name="gpool", bufs=3) as gpool:
            ew_t = cpool.tile([P, total_cols], bf16)
            nc.sync.dma_start(out=ew_t[:], in_=ew_d[:])
            w_t = cpool.tile([P, TILES], f32)
            nc.sync.dma_start(out=w_t[:], in_=w_d[:])
            d2_t = cpool.tile([P, TILES], f32)
            nc.sync.dma_start(out=d2_t[:], in_=d2_d[:])
            h_t = cpool.tile([P, TILES * C], f32)
            nc.sync.dma_start(out=g3(h_t), in_=h0_d[:])
            g_t = cpool.tile([P, TILES * C], f32)
            gb_t = cpool.tile([P, TILES * C], bf16)
            agg_t = cpool.tile([P, TILES * C], f32)
            rs_t = cpool.tile([P, TILES], f32)

            def exchange():
                # push local g to every core's strided gather table:
                # AllGather (compact) then expand to 256B-strided rows,
                # one copy per bank (rows r = 4l+b) so bank-b gathers
                # only wait on their own expand; per-bank copies also
                # keep AP dims under the 16-bit limit
                nc.gpsimd.collective_compute(
                    "AllGather", AO.bypass,
                    ins=[gslice_d[:]], outs=[gfull_c[:]],
                    replica_groups=rg)
                for b in range(NBANK):
                    nc.sync.dma_start(
                        out=bass.AP(gfull_s[:].tensor, b * ROWW_E,
                                    [[NBANK * ROWW_E, NB_ROWS], [1, C]]),
                        in_=bass.AP(gfull_c[:].tensor, b * C,
                                    [[NBANK * C, NB_ROWS], [1, C]]))

            gsl = gslice_d[:].rearrange("(p x) c -> p (x c)", p=P)
            nc.vector.tensor_tensor(
                out=g3(gb_t), in0=g3(h_t), in1=bcast_last(w_t[:], C),
                op=AO.mult)
            nc.sync.dma_start(out=gsl, in_=gb_t[:])
            exchange()

            for layer in range(LAYERS):
                colb = 0
                idxb = 0
                for ci, (t0, nt, kb) in enumerate(chunks):
                    W = sum(kb)
                    icols = idx_cols[ci]
                    idx_t = gpool.tile([P, max_idx_cols], mybir.dt.int16,
                                       tag="idx")
                    nc.sync.dma_start(out=idx_t[:, :icols],
                                      in_=gi_d[:, idxb:idxb + icols])
                    msgs = gpool.tile([P, max_cols * C], bf16, tag="msgs")
                    msg2 = gpool.tile([P, max_cols * C], bf16, tag="msg2")
                    cb = 0
                    ib = 0
                    offb = 0
                    for b in range(NBANK):
                        if kb[b] == 0:
                            continue
                        ncols = nt * kb[b]
                        nidx = 128 * ncols
                        src = bass.AP(gfull_s[:].tensor, b * ROWW_E,
                                      [[NBANK * ROWW_E, NB_ROWS], [1, C]])
                        _dma_gather(
                            nc.gpsimd,
                            out_ap=msgs[:, cb * C:(cb + ncols) * C]
                                .rearrange("p (s c) -> p s c", c=C),
                            in_ap=src,
                            idxs_ap=idx_t[:, ib:ib + nidx // 16],
                            num_idxs=nidx, elem_size=C,
                            elem_step=NBANK * ROWW_E)
                        # weight and transpose bank-major -> tile-major so
                        # one reduce per chunk covers all banks
                        pstep = msg2[:].ap[0][0]
                        nc.vector.tensor_tensor(
                            out=bass.AP(msg2[:].tensor,
                                        msg2[:].offset + offb * C,
                                        [[pstep, P], [W * C, nt],
                                         [C, kb[b]], [1, C]]),
                            in0=msgs[:, cb * C:(cb + ncols) * C].rearrange(
                                "p (t k c) -> p t k c", k=kb[b], c=C),
                            in1=bcast_last(
                                ew_t[:, colb + cb:colb + cb + ncols]
                                .rearrange("p (t k) -> p t k", k=kb[b]), C),
                            op=AO.mult)
                        cb += ncols
                        ib += nidx // 16
                        offb += kb[b]
                    nc.vector.tensor_reduce(
                        out=agg_t[:, t0 * C:(t0 + nt) * C],
                        in_=msg2[:, :nt * W * C].rearrange(
                            "p (t k c) -> p t c k", k=W, c=C),
                        op=AO.add, axis=mybir.AxisListType.X)
                    colb += W * nt
                    idxb += icols

                    # per-chunk epilogue for tiles [t0, t0+nt): the
                    # h-update, next layer's g, and its gslice range all
                    # land while later chunks are still gathering, so
                    # only AllGather+expand stay serial between layers
                    sl = slice(t0 * C, (t0 + nt) * C)
                    ts = slice(t0, t0 + nt)

                    def s3(t):
                        return t[:, sl].rearrange("p (t c) -> p t c", c=C)

                    # d2 holds (1-ALPHA)*degree^2
                    nc.vector.tensor_tensor(
                        out=s3(g_t), in0=s3(agg_t),
                        in1=bcast_last(d2_t[:, ts], C), op=AO.mult)
                    nc.vector.scalar_tensor_tensor(
                        out=g_t[:, sl], in0=h_t[:, sl], scalar=ALPHA,
                        in1=g_t[:, sl], op0=AO.mult, op1=AO.add)
                    nc.vector.tensor_reduce(
                        out=rs_t[:, ts], in_=s3(g_t), op=AO.add,
                        axis=mybir.AxisListType.X)
                    nc.vector.reciprocal(out=rs_t[:, ts], in_=rs_t[:, ts])
                    nc.vector.tensor_tensor(
                        out=s3(h_t), in0=s3(g_t),
                        in1=bcast_last(rs_t[:, ts], C), op=AO.mult)
                    if layer < LAYERS - 1:
                        nc.vector.tensor_tensor(
                            out=gb_t[:, sl].rearrange(
                                "p (t c) -> p t c", c=C),
                            in0=s3(h_t),
                            in1=bcast_last(w_t[:, ts], C), op=AO.mult)
                        nc.sync.dma_start(out=gsl[:, sl], in_=gb_t[:, sl])

                if layer < LAYERS - 1:
                    exchange()

            nc.sync.dma_start(out=hout_d[:], in_=g3(h_t))

    nc.compile()
    return nc, chunks


def _prep(x, W, edge_weight, degree, edge_index):
    src = edge_index[0].astype(np.int64)
    dst = edge_index[1].astype(np.int64)
    indeg = np.bincount(dst, minlength=N)

    pos_of = np.empty(N, dtype=np.int64)
    core_of = np.empty(N, dtype=np.int64)
    for cc in range(N_CORES):
        ids = np.arange(cc * 12500, (cc + 1) * 12500)
        order = ids[np.argsort(-indeg[ids], kind="stable")]
        core_of[order] = cc
        pos_of[order] = np.arange(12500)
    tile_of = pos_of // P
    part_of = pos_of % P
    row_of = core_of * PER_CORE + part_of * TILES + tile_of

    bank_of_row = row_of % NBANK
    loc_of_row = row_of // NBANK

    # per (core, part, tile, bank) counts -> K_tb = max over cores+parts
    eb = bank_of_row[src]
    key = ((core_of[dst] * P + part_of[dst]) * TILES + tile_of[dst]) * NBANK + eb
    cnt = np.bincount(key, minlength=N_CORES * P * TILES * NBANK)
    cnt = cnt.reshape(N_CORES, P, TILES, NBANK)
    K_tb = cnt.max(axis=(0, 1))                      # [TILES, NBANK]

    chunks = _plan(K_tb)
    # column base for (t, b): columns laid chunk-major, bank-major inside
    col_of = np.zeros((TILES, NBANK), dtype=np.int64)
    kb_of = np.zeros((TILES, NBANK), dtype=np.int64)
    colb = 0
    for (t0, nt, kb) in chunks:
        cb = colb
        for b in range(NBANK):
            for t in range(t0, t0 + nt):
                col_of[t, b] = cb + (t - t0) * kb[b]
                kb_of[t, b] = kb[b]
            cb += nt * kb[b]
        colb += nt * sum(kb)
    total_cols = colb

    # slot position for each edge: within-group rank
    order = np.argsort(key, kind="stable")
    ks = key[order]
    start = np.zeros(key.max() + 2, dtype=np.int64)
    cnts = np.bincount(ks)
    start[1:len(cnts) + 1] = np.cumsum(cnts)
    kslot = np.arange(len(ks)) - start[ks]

    ec, ep = core_of[dst[order]], part_of[dst[order]]
    et, ebk = tile_of[dst[order]], eb[order]
    col = col_of[et, ebk] + kslot

    import ml_dtypes
    ew_h = np.zeros((N_CORES, P, total_cols), dtype=ml_dtypes.bfloat16)
    iv_h = np.zeros((N_CORES, P, total_cols), dtype=np.int16)
    ew_h[ec, ep, col] = edge_weight[order].astype(ml_dtypes.bfloat16)
    iv_h[ec, ep, col] = loc_of_row[src[order]].astype(np.int16)

    # wrapped int16 idx tensors, concatenated per chunk/bank region
    gi_list = []
    for (t0, nt, kb) in chunks:
        for b in range(NBANK):
            if kb[b] == 0:
                continue
            c0 = col_of[t0, b]
            ncols = nt * kb[b]
            vals = iv_h[:, :, c0:c0 + ncols]             # [NC, P, ncols]
            # position i = col*128 + p ; wrapped [q, m] = value(m*16 + q%16)
            flat = vals.transpose(0, 2, 1).reshape(N_CORES, ncols * 128)
            wr = flat.reshape(N_CORES, ncols * 8, 16).transpose(0, 2, 1)
            wr = np.tile(wr, (1, 8, 1))                  # replicate to 128
            gi_list.append(wr)
    gi = np.concatenate(gi_list, axis=2)

    h0 = np.ones((N_CORES, P, TILES, C), dtype=np.float32)
    wv = np.zeros((N_CORES, P, TILES), dtype=np.float32)
    d2 = np.zeros((N_CORES, P, TILES), dtype=np.float32)
    h0[core_of, part_of, tile_of] = np.asarray(x)
    wv[core_of, part_of, tile_of] = np.asarray(W).reshape(-1)
    d2[core_of, part_of, tile_of] = (1.0 - ALPHA) * np.asarray(degree) ** 2
    return (K_tb, gi, ew_h, h0, wv, d2, core_of, part_of, tile_of,
            chunks, col_of, kb_of)


def kernel(x, W, edge_weight, degree, edge_index):
    global _compiled
    from concourse import bass_utils
    bass_utils.upload_artifacts = lambda tmpdir: "local://" + tmpdir

    pr = _prep(np.asarray(x), np.asarray(W), np.asarray(edge_weight),
               np.asarray(degree), np.asarray(edge_index))
    K_tb, gi, ew, h0, wv, d2, core_of, part_of, tile_of = pr[:9]

    key = tuple(int(v) for v in K_tb.ravel())
    if _compiled is None or _compiled[0] != key:
        _compiled = (key, _build(K_tb)[0])
    nc = _compiled[1]

    in_maps = [{"h0": h0[cc], "w": wv[cc], "d2": d2[cc],
                "gi": gi[cc], "ew": ew[cc]} for cc in range(N_CORES)]
    res = bass_utils.run_bass_kernel_spmd(
        nc, in_maps, core_ids=list(range(N_CORES)))
    houts = np.stack([res.results[cc]["hout"] for cc in range(N_CORES)])
    return houts[core_of, part_of, tile_of]
